# revision 8
# baseline (speedup 1.0000x reference)
"""Causal self-attention (B=4, T=2048, C=1024, H=16) on 8 trn2 NeuronCores.

Sharding: core c handles batch b = c//2 and head-group hg = c%2 (8 of the 16
heads, i.e. 512 of the 1024 channels).  Each core computes its heads' QKV
projections, causal attention, and a *partial* out-projection over its 512
channels; the host sums the two partial outputs per batch (the "all-reduce" of
the row-sharded out_proj, done in numpy) and the hg==0 core adds bo.

Device-side layout is "transposed space": the host passes x[b].T so that the
contraction dim (channels) sits on SBUF partitions for every matmul:
  qT/kT [d, t] = W_hg @ x^T       (PE: lhsT = W^T chunk, rhs = x^T chunk)
  scoresT [s, q] = kT s-tile vs qT q-chunk (softmax dim on partitions)
  softmax-over-s via exp (ACT, scale=1/8) + ones-augmented V matmul:
  attn_out^T [65, q] = v_aug^T @ exp  (row 64 accumulates l = sum_s exp)
  normalize via PE outer-product broadcast of 1/l across partitions
  y[t, e] partial = aT^T @ Wo_hg^T
Causality at 128-token granularity by skipping upper s-tiles; the 4 diagonal
s-tiles of each 512-wide q-chunk are masked post-exp with precomputed 0/1
masks (only 4 distinct [128, 512] masks exist).

All big matmuls run in fp32r (full PE rate vs 4x-slow fp32).  walrus requires
fp32r matmul operands to be *produced* as fp32r: engine-produced tiles are
simply typed fp32r (the engine rounds), DMA-fed tiles are typed fp32r
end-to-end and the host pre-rounds the data (zero/round low 12 mantissa bits).
"""

import sys

if "/opt/trn_rl_repo" not in sys.path:
    sys.path.insert(0, "/opt/trn_rl_repo")

from contextlib import ExitStack

import numpy as np

import concourse.bass as bass
import concourse.tile as tile
from concourse import mybir
from concourse.bass_utils import run_bass_kernel_spmd

F32 = mybir.dt.float32
FR = mybir.dt.float32r

B, T, C, H, HD = 4, 2048, 1024, 16, 64
HPC = 8            # heads per core
DC = HPC * HD      # channels per core = 512
NCORES = 8
NG = DC // 128     # 4 d-chunks of 128 (2 heads each)
NKC = C // 128     # 8 contraction chunks over C
NTC = T // 512     # 4 q/t-chunks of 512
NST = T // 128     # 16 s/t-tiles of 128

USE_FP32R = True   # fp32r = full-rate PE (1 cyc/row at N>=256) vs fp32 (4 cyc/row)
MDT = FR if USE_FP32R else F32


def round_fp32r(a):
    """Round fp32 to fp32r (11 explicit mantissa bits, low 12 bits zeroed)."""
    if not USE_FP32R:
        return np.ascontiguousarray(a, np.float32)
    bits = np.ascontiguousarray(a, np.float32).view(np.uint32)
    return (((bits.astype(np.uint64) + 0x800) & 0xFFFFF000)
            .astype(np.uint32).view(np.float32))


def build():
    """Build the single-core Bass program (SPMD: all 8 cores run it)."""
    nc = bass.Bass("TRN2", target_bir_lowering=False, debug=False)

    xT = nc.dram_tensor("xT", [NKC, 128, T], MDT, kind="ExternalInput")
    wq = nc.dram_tensor("wq", [NKC, 128, DC], MDT, kind="ExternalInput")
    wk = nc.dram_tensor("wk", [NKC, 128, DC], MDT, kind="ExternalInput")
    wv = nc.dram_tensor("wv", [NKC, 128, DC], MDT, kind="ExternalInput")
    wo = nc.dram_tensor("wo", [NG, 128, C], MDT, kind="ExternalInput")
    bqk = nc.dram_tensor("bqk", [128, NG, 2], F32, kind="ExternalInput")
    bvb = nc.dram_tensor("bvb", [128, DC], F32, kind="ExternalInput")
    bob = nc.dram_tensor("bob", [128, C], F32, kind="ExternalInput")
    msk = nc.dram_tensor("msk", [128, 4, 512], F32, kind="ExternalInput")
    y = nc.dram_tensor("y", [T, C], F32, kind="ExternalOutput")

    EXP = mybir.ActivationFunctionType.Exp

    with tile.TileContext(nc) as tc, ExitStack() as ctx:
        big = ctx.enter_context(tc.tile_pool(name="big", bufs=1))
        qT = big.tile([128, NG, T], MDT, tag="qT")   # [p, g, t], d = g*128+p
        kT = big.tile([128, NG, T], MDT, tag="kT")
        vv = big.tile([128, NST, HPC, HD + 1], MDT, tag="v")  # last col = 1.0

        # ---------------- phase 1: QKV projections ----------------
        with tc.tile_pool(name="ph1w", bufs=1) as ph1w, \
             tc.tile_pool(name="ph1x", bufs=2) as ph1x, \
             tc.tile_pool(name="ph1s", bufs=1) as ph1s, \
             tc.tile_pool(name="psum1", bufs=4, space="PSUM") as psum1:
            wq_sb = ph1w.tile([128, NKC, DC], MDT, tag="wq")
            wk_sb = ph1w.tile([128, NKC, DC], MDT, tag="wk")
            wv_sb = ph1w.tile([128, NKC, DC], MDT, tag="wv")
            bqk_sb = ph1s.tile([128, NG, 2], F32, tag="bqk")
            bvb_sb = ph1s.tile([128, DC], F32, tag="bvb")
            for k in range(NKC):
                nc.sync.dma_start(out=wq_sb[:, k], in_=wq[k])
                nc.sync.dma_start(out=wk_sb[:, k], in_=wk[k])
                nc.sync.dma_start(out=wv_sb[:, k], in_=wv[k])
            nc.sync.dma_start(out=bqk_sb, in_=bqk[:])
            nc.sync.dma_start(out=bvb_sb, in_=bvb[:])
            # memset doesn't accept fp32r APs; 1.0 is fp32r-exact, so poke
            # the same bytes through an fp32 view
            nc.vector.memset(vv[:, :, :, HD:HD + 1].bitcast(F32), 1.0)

            for tci in range(NTC):
                tsl = slice(tci * 512, (tci + 1) * 512)
                xt = ph1x.tile([128, NKC, 512], MDT, tag="xt")
                for k in range(NKC):
                    nc.sync.dma_start(out=xt[:, k], in_=xT[k, :, tsl])
                for (w_sb, bcol, dst) in ((wq_sb, 0, qT), (wk_sb, 1, kT)):
                    for g in range(NG):
                        ps = psum1.tile([128, 512], F32, tag="ps1")
                        for k in range(NKC):
                            nc.tensor.matmul(
                                ps,
                                w_sb[:, k, g * 128:(g + 1) * 128],
                                xt[:, k],
                                start=(k == 0), stop=(k == NKC - 1))
                        nc.vector.tensor_scalar_add(
                            dst[:, g, tsl], ps, bqk_sb[:, g, bcol:bcol + 1])
                for si in range(4):
                    st = tci * 4 + si
                    ps = psum1.tile([128, 512], F32, tag="ps1")
                    for k in range(NKC):
                        nc.tensor.matmul(
                            ps,
                            xt[:, k, si * 128:(si + 1) * 128],
                            wv_sb[:, k],
                            start=(k == 0), stop=(k == NKC - 1))
                    nc.vector.tensor_add(
                        vv[:, st, :, 0:HD],
                        ps.rearrange("p (h d) -> p h d", h=HPC),
                        bvb_sb.rearrange("p (h d) -> p h d", h=HPC))

        # ---------------- phases 2+3 ----------------
        with tc.tile_pool(name="late", bufs=1) as late:
            wo_sb = late.tile([128, NG, C], MDT, tag="wo")
            bob_sb = late.tile([128, C], F32, tag="bob")
            aT = late.tile([128, NG, T], MDT, tag="aT")
            for g in range(NG):
                nc.sync.dma_start(out=wo_sb[:, g], in_=wo[g])
            nc.sync.dma_start(out=bob_sb, in_=bob[:])

            # -------- phase 2: attention per head --------
            with tc.tile_pool(name="ph2s", bufs=1) as ph2s, \
                 tc.tile_pool(name="ph2e", bufs=2) as ph2e, \
                 tc.tile_pool(name="ph2t", bufs=2) as ph2t, \
                 tc.tile_pool(name="psum_s", bufs=2, space="PSUM") as psum_s, \
                 tc.tile_pool(name="psum_av", bufs=2, space="PSUM") as psum_av, \
                 tc.tile_pool(name="psum_bc", bufs=2, space="PSUM") as psum_bc:
                msk_sb = ph2s.tile([128, 4, 512], F32, tag="msk")
                ones_sb = ph2s.tile([128, HD], F32, tag="ones")
                nc.sync.dma_start(out=msk_sb, in_=msk[:])
                nc.vector.memset(ones_sb, 1.0)

                for h in range(HPC):
                    g, po = h // 2, (h % 2) * HD
                    for qc in range(NTC):
                        qs = slice(qc * 512, (qc + 1) * 512)
                        n_st = 4 * (qc + 1)
                        pav = psum_av.tile([HD + 1, 512], F32, tag="pav")

                        def emit_av(e_pair, pair):
                            for j in range(2):
                                st = pair * 2 + j
                                nc.tensor.matmul(
                                    pav,
                                    vv[:, st, h],
                                    e_pair[:, j],
                                    start=(st == 0), stop=(st == n_st - 1),
                                    skip_group_check=True)

                        pending = None
                        for pair in range(n_st // 2):
                            ps = psum_s.tile([128, 2, 512], F32, tag="ps2")
                            for j in range(2):
                                st = pair * 2 + j
                                nc.tensor.matmul(
                                    ps[:, j],
                                    kT[po:po + HD, g, st * 128:(st + 1) * 128],
                                    qT[po:po + HD, g, qs],
                                    start=True, stop=True,
                                    skip_group_check=True)
                            e = ph2e.tile([128, 2, 512], MDT, tag="e")
                            nc.scalar.activation(out=e, in_=ps, func=EXP, scale=0.125)
                            for j in range(2):
                                kk = pair * 2 + j - 4 * qc
                                if kk >= 0:
                                    nc.vector.tensor_mul(e[:, j], e[:, j], msk_sb[:, kk])
                            if pending is not None:
                                emit_av(*pending)
                            pending = (e, pair)
                        emit_av(*pending)

                        # normalize: aT[:, q] = pav[0:64, q] / pav[64, q]
                        lr = ph2t.tile([HD + 1, 512], F32, tag="lr")
                        nc.vector.tensor_copy(lr[HD:HD + 1], pav[HD:HD + 1])
                        nc.vector.reciprocal(lr[HD:HD + 1], lr[HD:HD + 1])
                        bc = psum_bc.tile([HD, 512], F32, tag="bc")
                        nc.tensor.matmul(bc, ones_sb[HD:HD + 1, :], lr[HD:HD + 1],
                                         start=True, stop=True, skip_group_check=True)
                        # DVE can read only ONE PSUM operand; stage bc in SBUF
                        bc_sb = ph2t.tile([HD, 512], F32, tag="bc_sb")
                        nc.vector.tensor_copy(bc_sb, bc)
                        if po == 0:
                            nc.vector.tensor_mul(aT[0:HD, g, qs], pav[0:HD], bc_sb)
                        else:
                            tmp = ph2t.tile([HD, 512], MDT, tag="tmp")
                            nc.vector.tensor_mul(tmp, pav[0:HD], bc_sb)
                            # DVE cannot shift partitions; DMA moves 0:64 -> 64:128
                            nc.sync.dma_start(out=aT[HD:128, g, qs], in_=tmp)

            # -------- phase 3: partial out-projection --------
            with tc.tile_pool(name="ph3", bufs=3) as ph3, \
                 tc.tile_pool(name="psum3", bufs=3, space="PSUM") as psum3:
                for tt in range(NST):
                    tsl = slice(tt * 128, (tt + 1) * 128)
                    po_ = psum3.tile([128, C], F32, tag="po")
                    for eh in range(2):
                        for g in range(NG):
                            nc.tensor.matmul(
                                po_[:, eh * 512:(eh + 1) * 512],
                                aT[:, g, tsl],
                                wo_sb[:, g, eh * 512:(eh + 1) * 512],
                                start=(g == 0), stop=(g == NG - 1),
                                skip_group_check=True)
                    ot = ph3.tile([128, C], F32, tag="ot")
                    nc.vector.tensor_add(ot, po_, bob_sb)
                    nc.sync.dma_start(out=y[tsl], in_=ot)

    _split_matmul_waits(nc)
    return nc


def _split_matmul_waits(nc):
    """walrus codegen allows only ONE sync-wait per engine instruction.
    Move surplus waits of any multi-wait instruction onto preceding
    same-engine NoOps (one wait each) — engine dispatch is in-order, so
    the NoOps gate the instruction."""
    from concourse import mybir

    inst_noop_cls = None
    for fn in nc.m.functions:
        for blk in fn.blocks:
            new_insts = []
            for inst in blk.instructions:
                si = getattr(inst, "sync_info", None)
                if (si is not None
                        and si.on_wait and len(si.on_wait) > 1):
                    if inst_noop_cls is None:
                        import bass_rust
                        inst_noop_cls = bass_rust.InstNoOp
                    waits = list(si.on_wait)
                    si.on_wait = waits[-1:]
                    for w in waits[:-1]:  # one wait per NoOp (HW limit)
                        nop = inst_noop_cls(
                            name=nc.get_next_instruction_name(), ins=[], outs=[])
                        nop.engine = inst.engine
                        nop.sync_info = mybir.SyncInfo(on_wait=[w], on_update=[])
                        new_insts.append(nop)
                new_insts.append(inst)
            blk.instructions[:] = new_insts


def prepare_inputs(inputs):
    """Per-core input maps (host-side sharding + layout munging)."""
    x = np.asarray(inputs["x"], np.float32)
    Wq = np.asarray(inputs["Wq"], np.float32)
    bq = np.asarray(inputs["bq"], np.float32)
    Wk = np.asarray(inputs["Wk"], np.float32)
    bk = np.asarray(inputs["bk"], np.float32)
    Wv = np.asarray(inputs["Wv"], np.float32)
    bv = np.asarray(inputs["bv"], np.float32)
    Wo = np.asarray(inputs["Wo"], np.float32)
    bo = np.asarray(inputs["bo"], np.float32)

    p = np.arange(128)[:, None, None]
    kk = np.arange(4)[None, :, None]
    f = np.arange(512)[None, None, :]
    msk = ((p + 128 * kk) <= f).astype(np.float32)  # [128, 4, 512]

    in_maps = []
    for c in range(NCORES):
        b, hg = c // 2, c % 2
        rows = slice(hg * DC, (hg + 1) * DC)
        in_maps.append({
            "xT": round_fp32r(x[b].T).reshape(NKC, 128, T),
            "wq": round_fp32r(Wq[rows, :].T).reshape(NKC, 128, DC),
            "wk": round_fp32r(Wk[rows, :].T).reshape(NKC, 128, DC),
            "wv": round_fp32r(Wv[rows, :].T).reshape(NKC, 128, DC),
            "wo": round_fp32r(Wo[:, rows].T).reshape(NG, 128, C),
            "bqk": np.ascontiguousarray(
                np.stack([bq[rows].reshape(NG, 128).T,
                          bk[rows].reshape(NG, 128).T], axis=-1)),
            "bvb": np.tile(bv[rows][None, :], (128, 1)),
            "bob": (np.tile(bo[None, :], (128, 1)) if hg == 0
                    else np.zeros((128, C), np.float32)),
            "msk": msk,
        })
    return in_maps


def gather_outputs(results):
    ys = [np.asarray(r["y"], np.float32) for r in results]
    return np.stack([ys[2 * b] + ys[2 * b + 1] for b in range(B)], axis=0)


def kernel(**inputs):
    nc = build()
    in_maps = prepare_inputs(inputs)
    res = run_bass_kernel_spmd(nc, in_maps, core_ids=list(range(NCORES)))
    return gather_outputs(res.results)


# revision 10
# speedup vs baseline: 20.1308x; 20.1308x over previous
"""Causal self-attention (B=4, T=2048, C=1024, H=16) on 8 trn2 NeuronCores.

Sharding: core c handles batch b = c//2 and head-group hg = c%2 (8 of the 16
heads, i.e. 512 of the 1024 channels).  Each core computes its heads' QKV
projections, causal attention, and a *partial* out-projection over its 512
channels; the host sums the two partial outputs per batch (the "all-reduce" of
the row-sharded out_proj, done in numpy) and the hg==0 core adds bo.

Device-side layout is "transposed space": the host passes x[b].T so that the
contraction dim (channels) sits on SBUF partitions for every matmul:
  qT/kT [d, t] = W_hg @ x^T       (PE: lhsT = W^T chunk, rhs = x^T chunk)
  scoresT [s, q] = kT s-tile vs qT q-chunk (softmax dim on partitions)
  softmax-over-s via exp (ACT, scale=1/8) + ones-augmented V matmul:
  attn_out^T [65, q] = v_aug^T @ exp  (row 64 accumulates l = sum_s exp)
  normalize via PE outer-product broadcast of 1/l across partitions
  y[t, e] partial = aT^T @ Wo_hg^T
Causality at 128-token granularity by skipping upper s-tiles; the 4 diagonal
s-tiles of each 512-wide q-chunk are masked post-exp with precomputed 0/1
masks (only 4 distinct [128, 512] masks exist).

All big matmuls run in fp32r (full PE rate vs 4x-slow fp32).  walrus requires
fp32r matmul operands to be *produced* as fp32r: engine-produced tiles are
simply typed fp32r (the engine rounds), DMA-fed tiles are typed fp32r
end-to-end and the host pre-rounds the data (zero/round low 12 mantissa bits).
"""

import sys

if "/opt/trn_rl_repo" not in sys.path:
    sys.path.insert(0, "/opt/trn_rl_repo")

from contextlib import ExitStack

import numpy as np

import concourse.bass as bass
import concourse.tile as tile
from concourse import mybir
from concourse.bass_utils import run_bass_kernel_spmd

F32 = mybir.dt.float32
FR = mybir.dt.float32r

B, T, C, H, HD = 4, 2048, 1024, 16, 64
HPC = 8            # heads per core
DC = HPC * HD      # channels per core = 512
NCORES = 8
NG = DC // 128     # 4 d-chunks of 128 (2 heads each)
NKC = C // 128     # 8 contraction chunks over C
NTC = T // 512     # 4 q/t-chunks of 512
NST = T // 128     # 16 s/t-tiles of 128

USE_FP32R = True   # fp32r = full-rate PE (1 cyc/row at N>=256) vs fp32 (4 cyc/row)
MDT = FR if USE_FP32R else F32


def round_fp32r(a):
    """Round fp32 to fp32r (11 explicit mantissa bits, low 12 bits zeroed)."""
    if not USE_FP32R:
        return np.ascontiguousarray(a, np.float32)
    bits = np.ascontiguousarray(a, np.float32).view(np.uint32)
    return (((bits.astype(np.uint64) + 0x800) & 0xFFFFF000)
            .astype(np.uint32).view(np.float32))


def build(reps=1):
    """Build the single-core Bass program (SPMD: all 8 cores run it).
    reps>1 repeats the whole body back-to-back in one NEFF (timing aid)."""
    nc = bass.Bass("TRN2", target_bir_lowering=False, debug=False)

    xT = nc.dram_tensor("xT", [NKC, 128, T], MDT, kind="ExternalInput")
    wq = nc.dram_tensor("wq", [NKC, 128, DC], MDT, kind="ExternalInput")
    wk = nc.dram_tensor("wk", [NKC, 128, DC], MDT, kind="ExternalInput")
    wv = nc.dram_tensor("wv", [NKC, 128, DC], MDT, kind="ExternalInput")
    wo = nc.dram_tensor("wo", [NG, 128, C], MDT, kind="ExternalInput")
    bqk = nc.dram_tensor("bqk", [128, NG, 2], F32, kind="ExternalInput")
    bvb = nc.dram_tensor("bvb", [128, DC], F32, kind="ExternalInput")
    bob = nc.dram_tensor("bob", [128, C], F32, kind="ExternalInput")
    msk = nc.dram_tensor("msk", [128, 4, 512], F32, kind="ExternalInput")
    y = nc.dram_tensor("y", [T, C], F32, kind="ExternalOutput")

    EXP = mybir.ActivationFunctionType.Exp

    with tile.TileContext(nc) as tc:
      for _rep in range(reps):
        with tc.tile_pool(name="big", bufs=1) as big:
          _emit_body(nc, tc, big, locals())

    _split_matmul_waits(nc)
    return nc


def _emit_body(nc, tc, big, env):
    xT, wq, wk, wv, wo = env["xT"], env["wq"], env["wk"], env["wv"], env["wo"]
    bqk, bvb, bob, msk, y = env["bqk"], env["bvb"], env["bob"], env["msk"], env["y"]
    EXP = env["EXP"]
    if True:
        qT = big.tile([128, NG, T], MDT, tag="qT")   # [p, g, t], d = g*128+p
        kT = big.tile([128, NG, T], MDT, tag="kT")
        vv = big.tile([128, NST, HPC, HD + 1], MDT, tag="v")  # last col = 1.0

        # ---------------- phase 1: QKV projections ----------------
        with tc.tile_pool(name="ph1w", bufs=1) as ph1w, \
             tc.tile_pool(name="ph1x", bufs=2) as ph1x, \
             tc.tile_pool(name="ph1s", bufs=1) as ph1s, \
             tc.tile_pool(name="psum1", bufs=4, space="PSUM") as psum1:
            wq_sb = ph1w.tile([128, NKC, DC], MDT, tag="wq")
            wk_sb = ph1w.tile([128, NKC, DC], MDT, tag="wk")
            wv_sb = ph1w.tile([128, NKC, DC], MDT, tag="wv")
            bqk_sb = ph1s.tile([128, NG, 2], F32, tag="bqk")
            bvb_sb = ph1s.tile([128, DC], F32, tag="bvb")
            for k in range(NKC):
                nc.sync.dma_start(out=wq_sb[:, k], in_=wq[k])
                nc.sync.dma_start(out=wk_sb[:, k], in_=wk[k])
                nc.sync.dma_start(out=wv_sb[:, k], in_=wv[k])
            nc.sync.dma_start(out=bqk_sb, in_=bqk[:])
            nc.sync.dma_start(out=bvb_sb, in_=bvb[:])
            # memset doesn't accept fp32r APs; 1.0 is fp32r-exact, so poke
            # the same bytes through an fp32 view
            nc.vector.memset(vv[:, :, :, HD:HD + 1].bitcast(F32), 1.0)

            for tci in range(NTC):
                tsl = slice(tci * 512, (tci + 1) * 512)
                xt = ph1x.tile([128, NKC, 512], MDT, tag="xt")
                for k in range(NKC):
                    nc.sync.dma_start(out=xt[:, k], in_=xT[k, :, tsl])
                for (w_sb, bcol, dst) in ((wq_sb, 0, qT), (wk_sb, 1, kT)):
                    for g in range(NG):
                        ps = psum1.tile([128, 512], F32, tag="ps1")
                        for k in range(NKC):
                            nc.tensor.matmul(
                                ps,
                                w_sb[:, k, g * 128:(g + 1) * 128],
                                xt[:, k],
                                start=(k == 0), stop=(k == NKC - 1))
                        nc.vector.tensor_scalar_add(
                            dst[:, g, tsl], ps, bqk_sb[:, g, bcol:bcol + 1])
                for si in range(4):
                    st = tci * 4 + si
                    ps = psum1.tile([128, 512], F32, tag="ps1")
                    for k in range(NKC):
                        nc.tensor.matmul(
                            ps,
                            xt[:, k, si * 128:(si + 1) * 128],
                            wv_sb[:, k],
                            start=(k == 0), stop=(k == NKC - 1))
                    nc.vector.tensor_add(
                        vv[:, st, :, 0:HD],
                        ps.rearrange("p (h d) -> p h d", h=HPC),
                        bvb_sb.rearrange("p (h d) -> p h d", h=HPC))

        # ---------------- phases 2+3 ----------------
        with tc.tile_pool(name="late", bufs=1) as late:
            wo_sb = late.tile([128, NG, C], MDT, tag="wo")
            bob_sb = late.tile([128, C], F32, tag="bob")
            aT = late.tile([128, NG, T], MDT, tag="aT")
            for g in range(NG):
                nc.sync.dma_start(out=wo_sb[:, g], in_=wo[g])
            nc.sync.dma_start(out=bob_sb, in_=bob[:])

            # -------- phase 2: attention per head --------
            with tc.tile_pool(name="ph2s", bufs=1) as ph2s, \
                 tc.tile_pool(name="ph2e", bufs=2) as ph2e, \
                 tc.tile_pool(name="ph2t", bufs=2) as ph2t, \
                 tc.tile_pool(name="psum_s", bufs=2, space="PSUM") as psum_s, \
                 tc.tile_pool(name="psum_av", bufs=2, space="PSUM") as psum_av, \
                 tc.tile_pool(name="psum_bc", bufs=2, space="PSUM") as psum_bc:
                msk_sb = ph2s.tile([128, 4, 512], F32, tag="msk")
                ones_sb = ph2s.tile([128, HD], F32, tag="ones")
                nc.sync.dma_start(out=msk_sb, in_=msk[:])
                nc.vector.memset(ones_sb, 1.0)

                for h in range(HPC):
                    g, po = h // 2, (h % 2) * HD
                    for qc in range(NTC):
                        qs = slice(qc * 512, (qc + 1) * 512)
                        n_st = 4 * (qc + 1)
                        pav = psum_av.tile([HD + 1, 512], F32, tag="pav")

                        def emit_av(e_pair, pair):
                            for j in range(2):
                                st = pair * 2 + j
                                nc.tensor.matmul(
                                    pav,
                                    vv[:, st, h],
                                    e_pair[:, j],
                                    start=(st == 0), stop=(st == n_st - 1),
                                    skip_group_check=True)

                        pending = None
                        for pair in range(n_st // 2):
                            ps = psum_s.tile([128, 2, 512], F32, tag="ps2")
                            for j in range(2):
                                st = pair * 2 + j
                                nc.tensor.matmul(
                                    ps[:, j],
                                    kT[po:po + HD, g, st * 128:(st + 1) * 128],
                                    qT[po:po + HD, g, qs],
                                    start=True, stop=True,
                                    skip_group_check=True)
                            e = ph2e.tile([128, 2, 512], MDT, tag="e")
                            nc.scalar.activation(out=e, in_=ps, func=EXP, scale=0.125)
                            for j in range(2):
                                kk = pair * 2 + j - 4 * qc
                                if kk >= 0:
                                    nc.vector.tensor_mul(e[:, j], e[:, j], msk_sb[:, kk])
                            if pending is not None:
                                emit_av(*pending)
                            pending = (e, pair)
                        emit_av(*pending)

                        # normalize: aT[:, q] = pav[0:64, q] / pav[64, q]
                        lr = ph2t.tile([HD + 1, 512], F32, tag="lr")
                        nc.vector.tensor_copy(lr[HD:HD + 1], pav[HD:HD + 1])
                        nc.vector.reciprocal(lr[HD:HD + 1], lr[HD:HD + 1])
                        bc = psum_bc.tile([HD, 512], F32, tag="bc")
                        nc.tensor.matmul(bc, ones_sb[HD:HD + 1, :], lr[HD:HD + 1],
                                         start=True, stop=True, skip_group_check=True)
                        # DVE can read only ONE PSUM operand; stage bc in SBUF
                        bc_sb = ph2t.tile([HD, 512], F32, tag="bc_sb")
                        nc.vector.tensor_copy(bc_sb, bc)
                        if po == 0:
                            nc.vector.tensor_mul(aT[0:HD, g, qs], pav[0:HD], bc_sb)
                        else:
                            tmp = ph2t.tile([HD, 512], MDT, tag="tmp")
                            nc.vector.tensor_mul(tmp, pav[0:HD], bc_sb)
                            # DVE cannot shift partitions; DMA moves 0:64 -> 64:128
                            nc.sync.dma_start(out=aT[HD:128, g, qs], in_=tmp)

            # -------- phase 3: partial out-projection --------
            with tc.tile_pool(name="ph3", bufs=3) as ph3, \
                 tc.tile_pool(name="psum3", bufs=3, space="PSUM") as psum3:
                for tt in range(NST):
                    tsl = slice(tt * 128, (tt + 1) * 128)
                    po_ = psum3.tile([128, C], F32, tag="po")
                    for eh in range(2):
                        for g in range(NG):
                            nc.tensor.matmul(
                                po_[:, eh * 512:(eh + 1) * 512],
                                aT[:, g, tsl],
                                wo_sb[:, g, eh * 512:(eh + 1) * 512],
                                start=(g == 0), stop=(g == NG - 1),
                                skip_group_check=True)
                    ot = ph3.tile([128, C], F32, tag="ot")
                    nc.vector.tensor_add(ot, po_, bob_sb)
                    nc.sync.dma_start(out=y[tsl], in_=ot)


def _split_matmul_waits(nc):
    """walrus codegen allows only ONE sync-wait per engine instruction.
    Move surplus waits of any multi-wait instruction onto preceding
    same-engine NoOps (one wait each) — engine dispatch is in-order, so
    the NoOps gate the instruction."""
    from concourse import mybir

    inst_noop_cls = None
    for fn in nc.m.functions:
        for blk in fn.blocks:
            new_insts = []
            for inst in blk.instructions:
                si = getattr(inst, "sync_info", None)
                if (si is not None
                        and si.on_wait and len(si.on_wait) > 1):
                    if inst_noop_cls is None:
                        import bass_rust
                        inst_noop_cls = bass_rust.InstNoOp
                    waits = list(si.on_wait)
                    si.on_wait = waits[-1:]
                    for w in waits[:-1]:  # one wait per NoOp (HW limit)
                        nop = inst_noop_cls(
                            name=nc.get_next_instruction_name(), ins=[], outs=[])
                        nop.engine = inst.engine
                        nop.sync_info = mybir.SyncInfo(on_wait=[w], on_update=[])
                        nc.register_instruction(nop)
                        new_insts.append(nop)
                new_insts.append(inst)
            blk.instructions[:] = new_insts


def prepare_inputs(inputs):
    """Per-core input maps (host-side sharding + layout munging)."""
    x = np.asarray(inputs["x"], np.float32)
    Wq = np.asarray(inputs["Wq"], np.float32)
    bq = np.asarray(inputs["bq"], np.float32)
    Wk = np.asarray(inputs["Wk"], np.float32)
    bk = np.asarray(inputs["bk"], np.float32)
    Wv = np.asarray(inputs["Wv"], np.float32)
    bv = np.asarray(inputs["bv"], np.float32)
    Wo = np.asarray(inputs["Wo"], np.float32)
    bo = np.asarray(inputs["bo"], np.float32)

    p = np.arange(128)[:, None, None]
    kk = np.arange(4)[None, :, None]
    f = np.arange(512)[None, None, :]
    msk = ((p + 128 * kk) <= f).astype(np.float32)  # [128, 4, 512]

    in_maps = []
    for c in range(NCORES):
        b, hg = c // 2, c % 2
        rows = slice(hg * DC, (hg + 1) * DC)
        in_maps.append({
            "xT": round_fp32r(x[b].T).reshape(NKC, 128, T),
            "wq": round_fp32r(Wq[rows, :].T).reshape(NKC, 128, DC),
            "wk": round_fp32r(Wk[rows, :].T).reshape(NKC, 128, DC),
            "wv": round_fp32r(Wv[rows, :].T).reshape(NKC, 128, DC),
            "wo": round_fp32r(Wo[:, rows].T).reshape(NG, 128, C),
            "bqk": np.ascontiguousarray(
                np.stack([bq[rows].reshape(NG, 128).T,
                          bk[rows].reshape(NG, 128).T], axis=-1)),
            "bvb": np.tile(bv[rows][None, :], (128, 1)),
            "bob": (np.tile(bo[None, :], (128, 1)) if hg == 0
                    else np.zeros((128, C), np.float32)),
            "msk": msk,
        })
    return in_maps


def gather_outputs(results):
    ys = [np.asarray(r["y"], np.float32) for r in results]
    return np.stack([ys[2 * b] + ys[2 * b + 1] for b in range(B)], axis=0)


def kernel(**inputs):
    nc = build()
    in_maps = prepare_inputs(inputs)
    res = run_bass_kernel_spmd(nc, in_maps, core_ids=list(range(NCORES)))
    return gather_outputs(res.results)


# revision 11
# speedup vs baseline: 161.6036x; 8.0277x over previous
"""Causal self-attention (B=4, T=2048, C=1024, H=16) on 8 trn2 NeuronCores.

Sharding: core c handles batch b = c//2 and head-group hg = c%2 (8 of the 16
heads, i.e. 512 of the 1024 channels).  Each core computes its heads' QKV
projections, causal attention, and a *partial* out-projection over its 512
channels; the host sums the two partial outputs per batch (the "all-reduce" of
the row-sharded out_proj, done in numpy) and the hg==0 core adds bo.

Device-side layout is "transposed space": the host passes x[b].T so that the
contraction dim (channels) sits on SBUF partitions for every matmul:
  qT/kT [d, t] = W_hg @ x^T       (PE: lhsT = W^T chunk, rhs = x^T chunk)
  scoresT [s, q] = kT s-tile vs qT q-chunk (softmax dim on partitions)
  softmax-over-s via exp (ACT, scale=1/8) + ones-augmented V matmul:
  attn_out^T [65, q] = v_aug^T @ exp  (row 64 accumulates l = sum_s exp)
  normalize via PE outer-product broadcast of 1/l across partitions
  y[t, e] partial = aT^T @ Wo_hg^T
Causality at 128-token granularity by skipping upper s-tiles; the 4 diagonal
s-tiles of each 512-wide q-chunk are masked post-exp with precomputed 0/1
masks (only 4 distinct [128, 512] masks exist).

All big matmuls run in fp32r (full PE rate vs 4x-slow fp32).  walrus requires
fp32r matmul operands to be *produced* as fp32r: engine-produced tiles are
simply typed fp32r (the engine rounds), DMA-fed tiles are typed fp32r
end-to-end and the host pre-rounds the data (zero/round low 12 mantissa bits).
"""

import sys

if "/opt/trn_rl_repo" not in sys.path:
    sys.path.insert(0, "/opt/trn_rl_repo")

from contextlib import ExitStack

import numpy as np

import concourse.bass as bass
import concourse.tile as tile
from concourse import mybir
from concourse.bass_utils import run_bass_kernel_spmd

F32 = mybir.dt.float32
FR = mybir.dt.float32r

DMA_LOADS = "gpsimd"   # engine for bulk input loads: "sync" or "gpsimd"
DMA_STORES = "sync"    # engine for output stores

B, T, C, H, HD = 4, 2048, 1024, 16, 64
HPC = 8            # heads per core
DC = HPC * HD      # channels per core = 512
NCORES = 8
NG = DC // 128     # 4 d-chunks of 128 (2 heads each)
NKC = C // 128     # 8 contraction chunks over C
NTC = T // 512     # 4 q/t-chunks of 512
NST = T // 128     # 16 s/t-tiles of 128

USE_FP32R = True   # fp32r = full-rate PE (1 cyc/row at N>=256) vs fp32 (4 cyc/row)
MDT = FR if USE_FP32R else F32


def round_fp32r(a):
    """Round fp32 to fp32r (11 explicit mantissa bits, low 12 bits zeroed)."""
    if not USE_FP32R:
        return np.ascontiguousarray(a, np.float32)
    bits = np.ascontiguousarray(a, np.float32).view(np.uint32)
    return (((bits.astype(np.uint64) + 0x800) & 0xFFFFF000)
            .astype(np.uint32).view(np.float32))


def build(reps=1):
    """Build the single-core Bass program (SPMD: all 8 cores run it).
    reps>1 repeats the whole body back-to-back in one NEFF (timing aid)."""
    nc = bass.Bass("TRN2", target_bir_lowering=False, debug=False)

    xT = nc.dram_tensor("xT", [NTC, NKC, 128, 512], MDT, kind="ExternalInput")
    wq = nc.dram_tensor("wq", [NKC, 128, DC], MDT, kind="ExternalInput")
    wk = nc.dram_tensor("wk", [NKC, 128, DC], MDT, kind="ExternalInput")
    wv = nc.dram_tensor("wv", [NKC, 128, DC], MDT, kind="ExternalInput")
    wo = nc.dram_tensor("wo", [NG, 128, C], MDT, kind="ExternalInput")
    bqk = nc.dram_tensor("bqk", [128, NG, 2], F32, kind="ExternalInput")
    bvb = nc.dram_tensor("bvb", [128, DC], F32, kind="ExternalInput")
    bob = nc.dram_tensor("bob", [128, C], F32, kind="ExternalInput")
    msk = nc.dram_tensor("msk", [128, 4, 512], F32, kind="ExternalInput")
    y = nc.dram_tensor("y", [T, C], F32, kind="ExternalOutput")

    EXP = mybir.ActivationFunctionType.Exp

    with tile.TileContext(nc) as tc:
      for _rep in range(reps):
        with tc.tile_pool(name="big", bufs=1) as big:
          _emit_body(nc, tc, big, locals())

    _split_matmul_waits(nc)
    return nc


def _emit_body(nc, tc, big, env):
    xT, wq, wk, wv, wo = env["xT"], env["wq"], env["wk"], env["wv"], env["wo"]
    bqk, bvb, bob, msk, y = env["bqk"], env["bvb"], env["bob"], env["msk"], env["y"]
    EXP = env["EXP"]
    if True:
        qT = big.tile([128, NG, T], MDT, tag="qT")   # [p, g, t], d = g*128+p
        kT = big.tile([128, NG, T], MDT, tag="kT")
        vv = big.tile([128, NST, HPC, HD + 1], MDT, tag="v")  # last col = 1.0

        # ---------------- phase 1: QKV projections ----------------
        with tc.tile_pool(name="ph1w", bufs=1) as ph1w, \
             tc.tile_pool(name="ph1x", bufs=2) as ph1x, \
             tc.tile_pool(name="ph1s", bufs=1) as ph1s, \
             tc.tile_pool(name="psum1", bufs=4, space="PSUM") as psum1:
            wq_sb = ph1w.tile([128, NKC, DC], MDT, tag="wq")
            wk_sb = ph1w.tile([128, NKC, DC], MDT, tag="wk")
            wv_sb = ph1w.tile([128, NKC, DC], MDT, tag="wv")
            bqk_sb = ph1s.tile([128, NG, 2], F32, tag="bqk")
            bvb_sb = ph1s.tile([128, DC], F32, tag="bvb")
            ld = getattr(nc, DMA_LOADS)
            for k in range(NKC):
                ld.dma_start(out=wq_sb[:, k], in_=wq[k])
                ld.dma_start(out=wk_sb[:, k], in_=wk[k])
                ld.dma_start(out=wv_sb[:, k], in_=wv[k])
            ld.dma_start(out=bqk_sb, in_=bqk[:])
            ld.dma_start(out=bvb_sb, in_=bvb[:])
            # memset doesn't accept fp32r APs; 1.0 is fp32r-exact, so poke
            # the same bytes through an fp32 view
            nc.vector.memset(vv[:, :, :, HD:HD + 1].bitcast(F32), 1.0)

            for tci in range(NTC):
                tsl = slice(tci * 512, (tci + 1) * 512)
                xt = ph1x.tile([128, NKC, 512], MDT, tag="xt")
                ld = getattr(nc, DMA_LOADS)
                for k in range(NKC):
                    ld.dma_start(out=xt[:, k], in_=xT[tci, k])
                for (w_sb, bcol, dst) in ((wq_sb, 0, qT), (wk_sb, 1, kT)):
                    for g in range(NG):
                        ps = psum1.tile([128, 512], F32, tag="ps1")
                        for k in range(NKC):
                            nc.tensor.matmul(
                                ps,
                                w_sb[:, k, g * 128:(g + 1) * 128],
                                xt[:, k],
                                start=(k == 0), stop=(k == NKC - 1))
                        nc.vector.tensor_scalar_add(
                            dst[:, g, tsl], ps, bqk_sb[:, g, bcol:bcol + 1])
                for si in range(4):
                    st = tci * 4 + si
                    ps = psum1.tile([128, 512], F32, tag="ps1")
                    for k in range(NKC):
                        nc.tensor.matmul(
                            ps,
                            xt[:, k, si * 128:(si + 1) * 128],
                            wv_sb[:, k],
                            start=(k == 0), stop=(k == NKC - 1))
                    nc.vector.tensor_add(
                        vv[:, st, :, 0:HD],
                        ps.rearrange("p (h d) -> p h d", h=HPC),
                        bvb_sb.rearrange("p (h d) -> p h d", h=HPC))

        # ---------------- phases 2+3 ----------------
        with tc.tile_pool(name="late", bufs=1) as late:
            wo_sb = late.tile([128, NG, C], MDT, tag="wo")
            bob_sb = late.tile([128, C], F32, tag="bob")
            aT = late.tile([128, NG, T], MDT, tag="aT")
            ld = getattr(nc, DMA_LOADS)
            for g in range(NG):
                ld.dma_start(out=wo_sb[:, g], in_=wo[g])
            ld.dma_start(out=bob_sb, in_=bob[:])

            # -------- phase 2: attention per head --------
            with tc.tile_pool(name="ph2s", bufs=1) as ph2s, \
                 tc.tile_pool(name="ph2e", bufs=2) as ph2e, \
                 tc.tile_pool(name="ph2t", bufs=2) as ph2t, \
                 tc.tile_pool(name="psum_s", bufs=2, space="PSUM") as psum_s, \
                 tc.tile_pool(name="psum_av", bufs=2, space="PSUM") as psum_av, \
                 tc.tile_pool(name="psum_bc", bufs=2, space="PSUM") as psum_bc:
                msk_sb = ph2s.tile([128, 4, 512], F32, tag="msk")
                ones_sb = ph2s.tile([128, HD], MDT, tag="ones")
                getattr(nc, DMA_LOADS).dma_start(out=msk_sb, in_=msk[:])
                nc.vector.memset(ones_sb.bitcast(F32), 1.0)

                for h in range(HPC):
                    g, po = h // 2, (h % 2) * HD
                    for qc in range(NTC):
                        qs = slice(qc * 512, (qc + 1) * 512)
                        n_st = 4 * (qc + 1)
                        pav = psum_av.tile([HD + 1, 512], F32, tag="pav")

                        def emit_av(e_pair, pair):
                            for j in range(2):
                                st = pair * 2 + j
                                nc.tensor.matmul(
                                    pav,
                                    vv[:, st, h],
                                    e_pair[:, j],
                                    start=(st == 0), stop=(st == n_st - 1),
                                    skip_group_check=True)

                        pending = None
                        for pair in range(n_st // 2):
                            ps = psum_s.tile([128, 2, 512], F32, tag="ps2")
                            for j in range(2):
                                st = pair * 2 + j
                                nc.tensor.matmul(
                                    ps[:, j],
                                    kT[po:po + HD, g, st * 128:(st + 1) * 128],
                                    qT[po:po + HD, g, qs],
                                    start=True, stop=True,
                                    skip_group_check=True)
                            e = ph2e.tile([128, 2, 512], MDT, tag="e")
                            nc.scalar.activation(out=e, in_=ps, func=EXP, scale=0.125)
                            for j in range(2):
                                kk = pair * 2 + j - 4 * qc
                                if kk >= 0:
                                    nc.vector.tensor_mul(e[:, j], e[:, j], msk_sb[:, kk])
                            if pending is not None:
                                emit_av(*pending)
                            pending = (e, pair)
                        emit_av(*pending)

                        # normalize: aT[:, q] = pav[0:64, q] / pav[64, q]
                        lr = ph2t.tile([HD + 1, 512], MDT, tag="lr")
                        nc.vector.reciprocal(lr[HD:HD + 1], pav[HD:HD + 1])
                        bc = psum_bc.tile([HD, 512], F32, tag="bc")
                        nc.tensor.matmul(bc, ones_sb[HD:HD + 1, :], lr[HD:HD + 1],
                                         start=True, stop=True, skip_group_check=True)
                        # DVE can read only ONE PSUM operand; stage bc in SBUF
                        bc_sb = ph2t.tile([HD, 512], F32, tag="bc_sb")
                        nc.vector.tensor_copy(bc_sb, bc)
                        if po == 0:
                            nc.vector.tensor_mul(aT[0:HD, g, qs], pav[0:HD], bc_sb)
                        else:
                            tmp = ph2t.tile([HD, 512], MDT, tag="tmp")
                            nc.vector.tensor_mul(tmp, pav[0:HD], bc_sb)
                            # DVE cannot shift partitions; DMA moves 0:64 -> 64:128
                            getattr(nc, DMA_STORES).dma_start(out=aT[HD:128, g, qs], in_=tmp)

            # -------- phase 3: partial out-projection --------
            with tc.tile_pool(name="ph3", bufs=3) as ph3, \
                 tc.tile_pool(name="psum3", bufs=3, space="PSUM") as psum3:
                for tt in range(NST):
                    tsl = slice(tt * 128, (tt + 1) * 128)
                    po_ = psum3.tile([128, C], F32, tag="po")
                    for eh in range(2):
                        for g in range(NG):
                            nc.tensor.matmul(
                                po_[:, eh * 512:(eh + 1) * 512],
                                aT[:, g, tsl],
                                wo_sb[:, g, eh * 512:(eh + 1) * 512],
                                start=(g == 0), stop=(g == NG - 1),
                                skip_group_check=True)
                    ot = ph3.tile([128, C], F32, tag="ot")
                    nc.vector.tensor_add(ot, po_, bob_sb)
                    getattr(nc, DMA_STORES).dma_start(out=y[tsl], in_=ot)


def _split_matmul_waits(nc):
    """walrus codegen allows only ONE sync-wait per engine instruction.
    Move surplus waits of any multi-wait instruction onto preceding
    same-engine NoOps (one wait each) — engine dispatch is in-order, so
    the NoOps gate the instruction."""
    from concourse import mybir

    inst_noop_cls = None
    for fn in nc.m.functions:
        for blk in fn.blocks:
            new_insts = []
            for inst in blk.instructions:
                si = getattr(inst, "sync_info", None)
                if (si is not None
                        and si.on_wait and len(si.on_wait) > 1):
                    if inst_noop_cls is None:
                        import bass_rust
                        inst_noop_cls = bass_rust.InstNoOp
                    waits = list(si.on_wait)
                    si.on_wait = waits[-1:]
                    for w in waits[:-1]:  # one wait per NoOp (HW limit)
                        nop = inst_noop_cls(
                            name=nc.get_next_instruction_name(), ins=[], outs=[])
                        nop.engine = inst.engine
                        nop.sync_info = mybir.SyncInfo(on_wait=[w], on_update=[])
                        nc.register_instruction(nop)
                        new_insts.append(nop)
                new_insts.append(inst)
            blk.instructions[:] = new_insts


def prepare_inputs(inputs):
    """Per-core input maps (host-side sharding + layout munging)."""
    x = np.asarray(inputs["x"], np.float32)
    Wq = np.asarray(inputs["Wq"], np.float32)
    bq = np.asarray(inputs["bq"], np.float32)
    Wk = np.asarray(inputs["Wk"], np.float32)
    bk = np.asarray(inputs["bk"], np.float32)
    Wv = np.asarray(inputs["Wv"], np.float32)
    bv = np.asarray(inputs["bv"], np.float32)
    Wo = np.asarray(inputs["Wo"], np.float32)
    bo = np.asarray(inputs["bo"], np.float32)

    p = np.arange(128)[:, None, None]
    kk = np.arange(4)[None, :, None]
    f = np.arange(512)[None, None, :]
    msk = ((p + 128 * kk) <= f).astype(np.float32)  # [128, 4, 512]

    in_maps = []
    for c in range(NCORES):
        b, hg = c // 2, c % 2
        rows = slice(hg * DC, (hg + 1) * DC)
        in_maps.append({
            "xT": np.ascontiguousarray(round_fp32r(x[b].T).reshape(NKC, 128, NTC, 512).transpose(2, 0, 1, 3)),
            "wq": round_fp32r(Wq[rows, :].T).reshape(NKC, 128, DC),
            "wk": round_fp32r(Wk[rows, :].T).reshape(NKC, 128, DC),
            "wv": round_fp32r(Wv[rows, :].T).reshape(NKC, 128, DC),
            "wo": round_fp32r(Wo[:, rows].T).reshape(NG, 128, C),
            "bqk": np.ascontiguousarray(
                np.stack([bq[rows].reshape(NG, 128).T,
                          bk[rows].reshape(NG, 128).T], axis=-1)),
            "bvb": np.tile(bv[rows][None, :], (128, 1)),
            "bob": (np.tile(bo[None, :], (128, 1)) if hg == 0
                    else np.zeros((128, C), np.float32)),
            "msk": msk,
        })
    return in_maps


def gather_outputs(results):
    ys = [np.asarray(r["y"], np.float32) for r in results]
    return np.stack([ys[2 * b] + ys[2 * b + 1] for b in range(B)], axis=0)


def kernel(**inputs):
    nc = build()
    in_maps = prepare_inputs(inputs)
    res = run_bass_kernel_spmd(nc, in_maps, core_ids=list(range(NCORES)))
    return gather_outputs(res.results)
